# revision 1
# baseline (speedup 1.0000x reference)
"""Trainium2 Bass kernel for ContextAwareArtRecSys (gnn_message_passing).

Math fold: the reference is
    score[e] = concat(z_u[src] @ Wu.T + bu, z_i[dst] @ Wi.T + bi) @ wo.T + bo
Everything after the gather is linear, so with
    vu = wo[:, :128] @ Wu          (256-vector)
    vi = wo[:, 128:] @ Wi          (256-vector)
    c  = wo[:, :128]@bu + wo[:, 128:]@bi + bo   (scalar)
we have score[e] = (z_u @ vu)[src] + (z_i @ vi)[dst] + c.

Device plan per core k of 8 (SPMD):
  1. fold vu/vi/c on PE from the raw weights (replicated inputs).
  2. matvec s_u = z_u_shard @ vu + c, s_i = z_i_shard @ vi on DVE while
     the z shards stream in (z is sharded 8-ways by node).
  3. AllGather the scalar tables to DRAM (50000 + 100000 f32).
  4. per-element indirect-DMA gather of s_u[src_e] and s_i[dst_e] for the
     core's 1/8 slice of the edge list, add, store.
All heavy work is on-device; the host only slices/casts/pads/lays out
shards and concatenates the output.
"""

import numpy as np

N_CORES = 8
N_USERS, N_ITEMS, E, H = 50000, 100000, 500000, 256
HALF = H // 2

U_SH = N_USERS // N_CORES          # 6250 users per core
I_SH = N_ITEMS // N_CORES          # 12500 items per core
E_SH = E // N_CORES                # 62500 edges per core
U_PAD = 6272                       # 49 * 128
I_PAD = 12544                      # 98 * 128
U_TILES = U_PAD // 128             # 49
I_TILES = I_PAD // 128             # 98

N_GI = 16                          # gather instructions per table
CP = 3968                          # elements gathered per instruction (31*128)
COLS = CP // 128                   # 31 idx columns per instruction
E_PAD = N_GI * CP                  # 63488 padded edge slots per core

_CACHE = {}


def _build():
    if "nc" in _CACHE:
        return _CACHE["nc"]
    import concourse.bass as bass
    import concourse.tile as tile
    import concourse.mybir as mybir
    from concourse import bacc
    from concourse.bass import IndirectOffsetOnAxis

    f32 = mybir.dt.float32
    i32 = mybir.dt.int32

    nc = bacc.Bacc("TRN2", target_bir_lowering=False, debug=False,
                   num_devices=N_CORES)

    zu = nc.dram_tensor("zu", [U_PAD, H], f32, kind="ExternalInput")
    zi = nc.dram_tensor("zi", [I_PAD, H], f32, kind="ExternalInput")
    w_user = nc.dram_tensor("w_user", [HALF, H], f32, kind="ExternalInput")
    w_item = nc.dram_tensor("w_item", [HALF, H], f32, kind="ExternalInput")
    wo_u = nc.dram_tensor("wo_u", [HALF, 1], f32, kind="ExternalInput")
    wo_i = nc.dram_tensor("wo_i", [HALF, 1], f32, kind="ExternalInput")
    b_user = nc.dram_tensor("b_user", [HALF, 1], f32, kind="ExternalInput")
    b_item = nc.dram_tensor("b_item", [HALF, 1], f32, kind="ExternalInput")
    b_out = nc.dram_tensor("b_out", [1, 1], f32, kind="ExternalInput")
    idxu = nc.dram_tensor("idxu", [128, N_GI * COLS], i32, kind="ExternalInput")
    idxi = nc.dram_tensor("idxi", [128, N_GI * COLS], i32, kind="ExternalInput")
    out = nc.dram_tensor("out", [N_GI, CP], f32, kind="ExternalOutput")

    s_uc = nc.dram_tensor("s_uc", [U_SH, 1], f32)
    s_ic = nc.dram_tensor("s_ic", [I_SH, 1], f32)
    s_uf = nc.dram_tensor("s_uf", [N_USERS, 1], f32, addr_space="Shared")
    s_if = nc.dram_tensor("s_if", [N_ITEMS, 1], f32, addr_space="Shared")

    groups = [list(range(N_CORES))]

    with tile.TileContext(nc) as tc:
        with (
            tc.tile_pool(name="consts", bufs=1) as consts,
            tc.tile_pool(name="zpool", bufs=4) as zpool,
            tc.tile_pool(name="spool", bufs=1) as spool,
            tc.tile_pool(name="gpool", bufs=1) as gpool,
            tc.tile_pool(name="psum", bufs=2, space="PSUM") as psum,
        ):
            # ---- fold vu / vi / c on PE ----
            wu_t = consts.tile([HALF, H], f32)
            nc.sync.dma_start(wu_t[:], w_user.ap())
            wi_t = consts.tile([HALF, H], f32)
            nc.sync.dma_start(wi_t[:], w_item.ap())
            wou_t = consts.tile([HALF, 1], f32)
            nc.sync.dma_start(wou_t[:], wo_u.ap())
            woi_t = consts.tile([HALF, 1], f32)
            nc.sync.dma_start(woi_t[:], wo_i.ap())
            bu_t = consts.tile([HALF, 1], f32)
            nc.sync.dma_start(bu_t[:], b_user.ap())
            bi_t = consts.tile([HALF, 1], f32)
            nc.sync.dma_start(bi_t[:], b_item.ap())
            bo_t = consts.tile([1, 1], f32)
            nc.sync.dma_start(bo_t[:], b_out.ap())

            # replicate wo halves across the free dim: rep[k, m] = wo[k]
            ones_kk = consts.tile([HALF, HALF], f32)
            nc.vector.memset(ones_kk[:], 1.0)
            wou_rep = consts.tile([HALF, HALF], f32)
            nc.vector.tensor_scalar_mul(wou_rep[:], ones_kk[:], wou_t[:])
            woi_rep = consts.tile([HALF, HALF], f32)
            nc.vector.tensor_scalar_mul(woi_rep[:], ones_kk[:], woi_t[:])

            # vu/vi broadcast across all 128 partitions: [128, H] PSUM
            vu_ps = psum.tile([HALF, H], f32, tag="vps")
            nc.tensor.matmul(vu_ps[:], wou_rep[:], wu_t[:], start=True, stop=True)
            vu_t = consts.tile([HALF, H], f32)
            nc.vector.tensor_copy(vu_t[:], vu_ps[:])
            vi_ps = psum.tile([HALF, H], f32, tag="vps")
            nc.tensor.matmul(vi_ps[:], woi_rep[:], wi_t[:], start=True, stop=True)
            vi_t = consts.tile([HALF, H], f32)
            nc.vector.tensor_copy(vi_t[:], vi_ps[:])

            # c = wo_u . b_user + wo_i . b_item + b_out, broadcast to [128,1]
            ones_k1 = consts.tile([HALF, 128], f32)
            nc.vector.memset(ones_k1[:], 1.0)
            cu_ps = psum.tile([128, 1], f32, tag="cps")
            bub = consts.tile([HALF, 128], f32)
            nc.vector.tensor_scalar_mul(bub[:], ones_k1[:], bu_t[:])
            bib = consts.tile([HALF, 128], f32)
            nc.vector.tensor_scalar_mul(bib[:], ones_k1[:], bi_t[:])
            # cu_ps[m, 0] = sum_k bub[k, m] * wo_u[k]  (same for all m)
            nc.tensor.matmul(cu_ps[:], bub[:], wou_t[:], start=True, stop=False)
            nc.tensor.matmul(cu_ps[:], bib[:], woi_t[:], start=False, stop=False)
            # += 1 * b_out via a K=1 matmul (ones row as lhsT)
            nc.tensor.matmul(
                cu_ps[:], ones_k1[0:1, :], bo_t[:], start=False, stop=True
            )
            c_t = consts.tile([128, 1], f32)
            nc.vector.tensor_copy(c_t[:], cu_ps[:])

            # ---- phase 1: s tables (DVE matvec over streamed z tiles) ----
            su_sb = spool.tile([128, U_TILES], f32)
            for t in range(U_TILES):
                zt = zpool.tile([128, H], f32, tag="z")
                nc.sync.dma_start(zt[:], zu.ap()[t * 128:(t + 1) * 128, :])
                pr = zpool.tile([128, H], f32, tag="prod")
                nc.vector.tensor_mul(pr[:], zt[:], vu_t[:])
                nc.vector.reduce_sum(
                    su_sb[:, t:t + 1], pr[:], axis=mybir.AxisListType.X
                )
            # add folded constant c into the user table
            nc.vector.tensor_scalar_add(su_sb[:], su_sb[:], c_t[:])

            # store + allgather the user table immediately so the user
            # gathers can start while the item table is still being built
            # s_uc[n] for n = 128*t + p  <=>  sbuf [p, t]
            n_full_u = U_SH // 128              # 48 full tiles
            rem_u = U_SH - n_full_u * 128       # 106
            nc.sync.dma_start(
                s_uc.ap()[: n_full_u * 128, :].rearrange(
                    "(t p) one -> p (t one)", p=128
                ),
                su_sb[:, :n_full_u],
            )
            nc.sync.dma_start(
                s_uc.ap()[n_full_u * 128:, :],
                su_sb[:rem_u, n_full_u:n_full_u + 1],
            )
            nc.gpsimd.collective_compute(
                "AllGather", mybir.AluOpType.bypass,
                replica_groups=groups, ins=[s_uc.ap()], outs=[s_uf.ap()],
            )

            si_sb = spool.tile([128, I_TILES], f32)
            for t in range(I_TILES):
                zt = zpool.tile([128, H], f32, tag="z")
                nc.sync.dma_start(zt[:], zi.ap()[t * 128:(t + 1) * 128, :])
                pr = zpool.tile([128, H], f32, tag="prod")
                nc.vector.tensor_mul(pr[:], zt[:], vi_t[:])
                nc.vector.reduce_sum(
                    si_sb[:, t:t + 1], pr[:], axis=mybir.AxisListType.X
                )

            # ---- store item chunk to DRAM (node-major order) ----
            n_full_i = I_SH // 128              # 97
            rem_i = I_SH - n_full_i * 128       # 84
            nc.sync.dma_start(
                s_ic.ap()[: n_full_i * 128, :].rearrange(
                    "(t p) one -> p (t one)", p=128
                ),
                si_sb[:, :n_full_i],
            )
            nc.sync.dma_start(
                s_ic.ap()[n_full_i * 128:, :],
                si_sb[:rem_i, n_full_i:n_full_i + 1],
            )

            nc.gpsimd.collective_compute(
                "AllGather", mybir.AluOpType.bypass,
                replica_groups=groups, ins=[s_ic.ap()], outs=[s_if.ap()],
            )

            # ---- phase 2: per-element indirect gathers ----
            idxu_t = gpool.tile([128, N_GI * COLS], i32)
            nc.sync.dma_start(idxu_t[:], idxu.ap())
            idxi_t = gpool.tile([128, N_GI * COLS], i32)
            nc.sync.dma_start(idxi_t[:], idxi.ap())

            # all user gathers FIRST: gpsimd runs in order, and the user
            # table is ready well before the item table. Each batch sits in
            # a critical section with a manual completion semaphore so the
            # Q7 generates descriptors back-to-back instead of waiting for
            # each gather's DMA to drain.
            gu_t = gpool.tile([128, CP], f32)
            gi_t = gpool.tile([128, CP], f32)
            for p in range(N_GI):
                nc.gpsimd.indirect_dma_start(
                    out=gu_t[p:p + 1, :].rearrange("one (c x) -> one c x", x=1),
                    out_offset=None,
                    in_=s_uf.ap(),
                    in_offset=IndirectOffsetOnAxis(
                        ap=idxu_t[:, p * COLS:(p + 1) * COLS], axis=0
                    ),
                )
            for p in range(N_GI):
                nc.gpsimd.indirect_dma_start(
                    out=gi_t[p:p + 1, :].rearrange("one (c x) -> one c x", x=1),
                    out_offset=None,
                    in_=s_if.ap(),
                    in_offset=IndirectOffsetOnAxis(
                        ap=idxi_t[:, p * COLS:(p + 1) * COLS], axis=0
                    ),
                )
            sc_t = gpool.tile([N_GI, CP], f32)
            nc.vector.tensor_add(sc_t[:], gu_t[:N_GI, :], gi_t[:N_GI, :])
            nc.sync.dma_start(out.ap(), sc_t[:])

    nc.compile()
    _CACHE["nc"] = nc
    return nc


def _wrap_idx(vals):
    """Lay a per-core int32 index stream out for the gather instructions.

    Instruction p consumes its [128, COLS] idx slice in spray order
    (partition-fastest), writing element j of its row; so slice columns
    hold vals[p*CP : (p+1)*CP] reshaped (COLS, 128) transposed.
    """
    full = np.zeros(E_PAD, dtype=np.int32)
    full[: len(vals)] = vals
    outm = np.empty((128, N_GI * COLS), dtype=np.int32)
    for p in range(N_GI):
        seg = full[p * CP:(p + 1) * CP]
        outm[:, p * COLS:(p + 1) * COLS] = seg.reshape(COLS, 128).T
    return outm


def _make_in_maps(inputs):
    z_user = np.ascontiguousarray(np.asarray(inputs["z_user"], dtype=np.float32))
    z_item = np.ascontiguousarray(np.asarray(inputs["z_item"], dtype=np.float32))
    src = np.asarray(inputs["edge_src"]).astype(np.int32)
    dst = np.asarray(inputs["edge_dst"]).astype(np.int32)
    w_user = np.asarray(inputs["w_user"], dtype=np.float32)
    w_item = np.asarray(inputs["w_item"], dtype=np.float32)
    b_user = np.asarray(inputs["b_user"], dtype=np.float32).reshape(HALF, 1)
    b_item = np.asarray(inputs["b_item"], dtype=np.float32).reshape(HALF, 1)
    w_out = np.asarray(inputs["w_out"], dtype=np.float32)
    b_out = np.asarray(inputs["b_out"], dtype=np.float32).reshape(1, 1)
    wo_u = w_out[0, :HALF].reshape(HALF, 1).copy()
    wo_i = w_out[0, HALF:].reshape(HALF, 1).copy()

    in_maps = []
    for k in range(N_CORES):
        zu_k = np.zeros((U_PAD, H), dtype=np.float32)
        zu_k[:U_SH] = z_user[k * U_SH:(k + 1) * U_SH]
        zi_k = np.zeros((I_PAD, H), dtype=np.float32)
        zi_k[:I_SH] = z_item[k * I_SH:(k + 1) * I_SH]
        in_maps.append({
            "zu": zu_k,
            "zi": zi_k,
            "w_user": w_user,
            "w_item": w_item,
            "wo_u": wo_u,
            "wo_i": wo_i,
            "b_user": b_user,
            "b_item": b_item,
            "b_out": b_out,
            "idxu": _wrap_idx(src[k * E_SH:(k + 1) * E_SH]),
            "idxi": _wrap_idx(dst[k * E_SH:(k + 1) * E_SH]),
        })
    return in_maps


def _run(inputs, trace=False):
    from concourse.bass_utils import run_bass_kernel_spmd

    nc = _build()
    in_maps = _make_in_maps(inputs)
    res = run_bass_kernel_spmd(
        nc, in_maps, core_ids=list(range(N_CORES)), trace=trace
    )
    parts = [res.results[k]["out"].reshape(-1)[:E_SH] for k in range(N_CORES)]
    full = np.concatenate(parts).reshape(E, 1).astype(np.float32)
    return full, res


def kernel(**inputs):
    full, _ = _run(inputs, trace=False)
    return full



# revision 10
# speedup vs baseline: 1.1953x; 1.1953x over previous
"""Trainium2 Bass kernel for ContextAwareArtRecSys (gnn_message_passing).

Math fold: the reference is
    score[e] = concat(z_u[src] @ Wu.T + bu, z_i[dst] @ Wi.T + bi) @ wo.T + bo
Everything after the gather is linear, so with
    vu = wo[:, :128] @ Wu          (256-vector)
    vi = wo[:, 128:] @ Wi          (256-vector)
    c  = wo[:, :128]@bu + wo[:, 128:]@bi + bo   (scalar)
we have score[e] = (z_u @ vu)[src] + (z_i @ vi)[dst] + c.

Device plan per core k of 8 (SPMD):
  1. fold vu/vi/c on PE from the raw weights (replicated inputs).
  2. matvec s_u = z_u_shard @ vu + c, s_i = z_i_shard @ vi on DVE while
     the z shards stream in as large (1.25 MB) DMAs from a host-swizzled
     [128, T*H] layout.
  3. AllGather the scalar tables to DRAM (50000 + 100000 f32).
  4. per-element indirect-DMA gather of s_u[src_e] and s_i[dst_e] for the
     core's 1/8 slice of the edge list. Each gather instruction writes one
     SBUF partition row; rows are spaced 8 apart so the 32 gathers' drains
     spread over all 16 SBUF AXI ports / SDMA engines, and the SWDGE
     descriptor ring is enlarged (32 KiB/partition) so one gather's 3968
     descriptors fit without backpressuring the Q7 generator.
  5. compact the strided rows with two SBUF->SBUF DMAs, add, store.
All heavy work is on-device; the host only slices/casts/pads/lays out
shards and concatenates the output.
"""

import numpy as np

N_CORES = 8
N_USERS, N_ITEMS, E, H = 50000, 100000, 500000, 256
HALF = H // 2

U_SH = N_USERS // N_CORES          # 6250 users per core
I_SH = N_ITEMS // N_CORES          # 12500 items per core
E_SH = E // N_CORES                # 62500 edges per core

U_TILES = 50                       # padded user row-tiles (6400 rows)
I_TILES = 100                      # padded item row-tiles (12800 rows)
CHUNK_T = 10                       # row-tiles per z DMA chunk (1.25 MB)
U_CHUNKS = U_TILES // CHUNK_T      # 5
I_CHUNKS = I_TILES // CHUNK_T      # 10

N_G = 16                           # gather instructions per table
GCOLS = 31                         # idx columns per gather (31*128 = 3968)
CP = GCOLS * 128                   # 3968 elements per gather instruction
COLS = N_G * GCOLS                 # 496 total idx columns
E_PAD = N_G * CP                   # 63488 padded edge slots per core

_CACHE = {}


def _build():
    if "nc" in _CACHE:
        return _CACHE["nc"]
    import concourse.bass as bass
    import concourse.tile as tile
    import concourse.mybir as mybir
    from concourse import bacc
    from concourse.bass import IndirectOffsetOnAxis

    f32 = mybir.dt.float32
    i32 = mybir.dt.int32

    nc = bacc.Bacc("TRN2", target_bir_lowering=False, debug=False,
                   num_devices=N_CORES, dynamic_dma_scratch_size=32768)

    zu = nc.dram_tensor("zu", [128, U_TILES * H], f32, kind="ExternalInput")
    zi = nc.dram_tensor("zi", [128, I_TILES * H], f32, kind="ExternalInput")
    w_user = nc.dram_tensor("w_user", [HALF, H], f32, kind="ExternalInput")
    w_item = nc.dram_tensor("w_item", [HALF, H], f32, kind="ExternalInput")
    wo_u = nc.dram_tensor("wo_u", [HALF, 1], f32, kind="ExternalInput")
    wo_i = nc.dram_tensor("wo_i", [HALF, 1], f32, kind="ExternalInput")
    b_user = nc.dram_tensor("b_user", [HALF, 1], f32, kind="ExternalInput")
    b_item = nc.dram_tensor("b_item", [HALF, 1], f32, kind="ExternalInput")
    b_out = nc.dram_tensor("b_out", [1, 1], f32, kind="ExternalInput")
    idx = nc.dram_tensor("idx", [128, 2 * COLS], i32, kind="ExternalInput")
    out = nc.dram_tensor("out", [N_G, CP], f32, kind="ExternalOutput")

    s_uc = nc.dram_tensor("s_uc", [U_SH, 1], f32)
    s_ic = nc.dram_tensor("s_ic", [I_SH, 1], f32)
    s_uf = nc.dram_tensor("s_uf", [N_USERS, 1], f32, addr_space="Shared")
    s_if = nc.dram_tensor("s_if", [N_ITEMS, 1], f32, addr_space="Shared")

    groups = [list(range(N_CORES))]

    with tile.TileContext(nc) as tc:
        with (
            tc.tile_pool(name="consts", bufs=1) as consts,
            tc.tile_pool(name="zpool", bufs=3) as zpool,
            tc.tile_pool(name="scpool", bufs=2) as scpool,
            tc.tile_pool(name="spool", bufs=1) as spool,
            tc.tile_pool(name="gpool", bufs=1) as gpool,
            tc.tile_pool(name="psum", bufs=2, space="PSUM") as psum,
        ):
            # ---- fold vu / vi / c on PE ----
            wu_t = consts.tile([HALF, H], f32)
            nc.sync.dma_start(wu_t[:], w_user.ap())
            wi_t = consts.tile([HALF, H], f32)
            nc.sync.dma_start(wi_t[:], w_item.ap())
            wou_t = consts.tile([HALF, 1], f32)
            nc.sync.dma_start(wou_t[:], wo_u.ap())
            woi_t = consts.tile([HALF, 1], f32)
            nc.sync.dma_start(woi_t[:], wo_i.ap())
            bu_t = consts.tile([HALF, 1], f32)
            nc.sync.dma_start(bu_t[:], b_user.ap())
            bi_t = consts.tile([HALF, 1], f32)
            nc.sync.dma_start(bi_t[:], b_item.ap())
            bo_t = consts.tile([1, 1], f32)
            nc.sync.dma_start(bo_t[:], b_out.ap())

            # load gather indices early so they are resident
            idx_t = gpool.tile([128, 2 * COLS], i32)
            nc.sync.dma_start(idx_t[:], idx.ap())

            # replicate wo halves across the free dim: rep[k, m] = wo[k]
            ones_kk = consts.tile([HALF, HALF], f32)
            nc.vector.memset(ones_kk[:], 1.0)
            wou_rep = consts.tile([HALF, HALF], f32)
            nc.vector.tensor_scalar_mul(wou_rep[:], ones_kk[:], wou_t[:])
            woi_rep = consts.tile([HALF, HALF], f32)
            nc.vector.tensor_scalar_mul(woi_rep[:], ones_kk[:], woi_t[:])

            # vu/vi broadcast across all 128 partitions: [128, H] PSUM
            vu_ps = psum.tile([HALF, H], f32, tag="vps")
            nc.tensor.matmul(vu_ps[:], wou_rep[:], wu_t[:], start=True, stop=True)
            vu_t = consts.tile([HALF, H], f32)
            nc.vector.tensor_copy(vu_t[:], vu_ps[:])
            vi_ps = psum.tile([HALF, H], f32, tag="vps")
            nc.tensor.matmul(vi_ps[:], woi_rep[:], wi_t[:], start=True, stop=True)
            vi_t = consts.tile([HALF, H], f32)
            nc.vector.tensor_copy(vi_t[:], vi_ps[:])

            # c = wo_u . b_user + wo_i . b_item + b_out, broadcast to [128,1]
            ones_k1 = consts.tile([HALF, 128], f32)
            nc.vector.memset(ones_k1[:], 1.0)
            cu_ps = psum.tile([128, 1], f32, tag="cps")
            bub = consts.tile([HALF, 128], f32)
            nc.vector.tensor_scalar_mul(bub[:], ones_k1[:], bu_t[:])
            bib = consts.tile([HALF, 128], f32)
            nc.vector.tensor_scalar_mul(bib[:], ones_k1[:], bi_t[:])
            nc.tensor.matmul(cu_ps[:], bub[:], wou_t[:], start=True, stop=False)
            nc.tensor.matmul(cu_ps[:], bib[:], woi_t[:], start=False, stop=False)
            nc.tensor.matmul(
                cu_ps[:], ones_k1[0:1, :], bo_t[:], start=False, stop=True
            )
            c_t = consts.tile([128, 1], f32)
            nc.vector.tensor_copy(c_t[:], cu_ps[:])

            # ---- phase 1: s_u table (DVE matvec over big z chunks) ----
            su_sb = spool.tile([128, U_TILES], f32)
            for ch in range(U_CHUNKS):
                zt = zpool.tile([128, CHUNK_T * H], f32, tag="z")
                nc.sync.dma_start(
                    zt[:], zu.ap()[:, ch * CHUNK_T * H:(ch + 1) * CHUNK_T * H]
                )
                for t in range(CHUNK_T):
                    pr = scpool.tile([128, H], f32, tag="scr")
                    nc.vector.tensor_mul(pr[:], zt[:, t * H:(t + 1) * H], vu_t[:])
                    nc.vector.reduce_sum(
                        su_sb[:, ch * CHUNK_T + t:ch * CHUNK_T + t + 1],
                        pr[:],
                        axis=mybir.AxisListType.X,
                    )
            # add folded constant c into the user table
            nc.vector.tensor_scalar_add(su_sb[:], su_sb[:], c_t[:])

            # store + allgather the user table immediately so the user
            # gathers can start while the item table is still being built
            n_full_u = U_SH // 128              # 48 full tiles
            rem_u = U_SH - n_full_u * 128       # 106
            nc.sync.dma_start(
                s_uc.ap()[: n_full_u * 128, :].rearrange(
                    "(t p) one -> p (t one)", p=128
                ),
                su_sb[:, :n_full_u],
            )
            nc.sync.dma_start(
                s_uc.ap()[n_full_u * 128:, :],
                su_sb[:rem_u, n_full_u:n_full_u + 1],
            )
            nc.gpsimd.collective_compute(
                "AllGather", mybir.AluOpType.bypass,
                replica_groups=groups, ins=[s_uc.ap()], outs=[s_uf.ap()],
            )

            # ---- phase 2: s_i table ----
            si_sb = spool.tile([128, I_TILES], f32)
            for ch in range(I_CHUNKS):
                zt = zpool.tile([128, CHUNK_T * H], f32, tag="z")
                nc.sync.dma_start(
                    zt[:], zi.ap()[:, ch * CHUNK_T * H:(ch + 1) * CHUNK_T * H]
                )
                for t in range(CHUNK_T):
                    pr = scpool.tile([128, H], f32, tag="scr")
                    nc.vector.tensor_mul(pr[:], zt[:, t * H:(t + 1) * H], vi_t[:])
                    nc.vector.reduce_sum(
                        si_sb[:, ch * CHUNK_T + t:ch * CHUNK_T + t + 1],
                        pr[:],
                        axis=mybir.AxisListType.X,
                    )

            # ---- user gathers (issued before the item AllGather so they
            # run while the item table is still in flight); gather g writes
            # partition row 8g so drains spread over all 16 AXI ports ----
            gu_t = gpool.tile([128, CP], f32)
            for g in range(N_G):
                nc.gpsimd.indirect_dma_start(
                    out=gu_t[8 * g:8 * g + 1, :].rearrange(
                        "one (c x) -> one c x", x=1
                    ),
                    out_offset=None,
                    in_=s_uf.ap(),
                    in_offset=IndirectOffsetOnAxis(
                        ap=idx_t[:, g * GCOLS:(g + 1) * GCOLS], axis=0
                    ),
                )

            # ---- item table store + allgather ----
            n_full_i = I_SH // 128              # 97
            rem_i = I_SH - n_full_i * 128       # 84
            nc.sync.dma_start(
                s_ic.ap()[: n_full_i * 128, :].rearrange(
                    "(t p) one -> p (t one)", p=128
                ),
                si_sb[:, :n_full_i],
            )
            nc.sync.dma_start(
                s_ic.ap()[n_full_i * 128:, :],
                si_sb[:rem_i, n_full_i:n_full_i + 1],
            )
            nc.gpsimd.collective_compute(
                "AllGather", mybir.AluOpType.bypass,
                replica_groups=groups, ins=[s_ic.ap()], outs=[s_if.ap()],
            )

            # ---- item gathers ----
            gi_t = gpool.tile([128, CP], f32)
            for g in range(N_G):
                nc.gpsimd.indirect_dma_start(
                    out=gi_t[8 * g:8 * g + 1, :].rearrange(
                        "one (c x) -> one c x", x=1
                    ),
                    out_offset=None,
                    in_=s_if.ap(),
                    in_offset=IndirectOffsetOnAxis(
                        ap=idx_t[:, COLS + g * GCOLS:COLS + (g + 1) * GCOLS],
                        axis=0,
                    ),
                )

            # compact strided rows 0,8,...,120 -> 16 contiguous partitions
            guc = gpool.tile([N_G, CP], f32)
            nc.sync.dma_start(guc[:], gu_t[0:128:8, :])
            gic = gpool.tile([N_G, CP], f32)
            nc.sync.dma_start(gic[:], gi_t[0:128:8, :])
            sc_t = gpool.tile([N_G, CP], f32)
            nc.vector.tensor_add(sc_t[:], guc[:], gic[:])
            nc.sync.dma_start(out.ap(), sc_t[:])

    nc.compile()
    _CACHE["nc"] = nc
    return nc


def _wrap_idx(vals):
    """Lay a per-core int32 index stream out for the gather instructions.

    Gather g writes its 3968 values contiguously (out slot e = g*CP + j)
    and consumes its [128, GCOLS] idx slice partition-fastest, so slice
    columns hold vals[g*CP:(g+1)*CP] reshaped (GCOLS, 128) transposed.
    """
    full = np.zeros(E_PAD, dtype=np.int32)
    full[: len(vals)] = vals
    outm = np.empty((128, COLS), dtype=np.int32)
    for g in range(N_G):
        seg = full[g * CP:(g + 1) * CP]
        outm[:, g * GCOLS:(g + 1) * GCOLS] = seg.reshape(GCOLS, 128).T
    return outm


def _swizzle_z(rows, tiles):
    """rows [n, H] -> [128, tiles*H] with column block t = rows[t*128:(t+1)*128]."""
    n = rows.shape[0]
    padded = np.zeros((tiles * 128, H), dtype=np.float32)
    padded[:n] = rows
    return np.ascontiguousarray(
        padded.reshape(tiles, 128, H).transpose(1, 0, 2).reshape(128, tiles * H)
    )


def _make_in_maps(inputs):
    z_user = np.asarray(inputs["z_user"], dtype=np.float32)
    z_item = np.asarray(inputs["z_item"], dtype=np.float32)
    src = np.asarray(inputs["edge_src"]).astype(np.int32)
    dst = np.asarray(inputs["edge_dst"]).astype(np.int32)
    w_user = np.asarray(inputs["w_user"], dtype=np.float32)
    w_item = np.asarray(inputs["w_item"], dtype=np.float32)
    b_user = np.asarray(inputs["b_user"], dtype=np.float32).reshape(HALF, 1)
    b_item = np.asarray(inputs["b_item"], dtype=np.float32).reshape(HALF, 1)
    w_out = np.asarray(inputs["w_out"], dtype=np.float32)
    b_out = np.asarray(inputs["b_out"], dtype=np.float32).reshape(1, 1)
    wo_u = w_out[0, :HALF].reshape(HALF, 1).copy()
    wo_i = w_out[0, HALF:].reshape(HALF, 1).copy()

    in_maps = []
    for k in range(N_CORES):
        idx_m = np.empty((128, 2 * COLS), dtype=np.int32)
        idx_m[:, :COLS] = _wrap_idx(src[k * E_SH:(k + 1) * E_SH])
        idx_m[:, COLS:] = _wrap_idx(dst[k * E_SH:(k + 1) * E_SH])
        in_maps.append({
            "zu": _swizzle_z(z_user[k * U_SH:(k + 1) * U_SH], U_TILES),
            "zi": _swizzle_z(z_item[k * I_SH:(k + 1) * I_SH], I_TILES),
            "w_user": w_user,
            "w_item": w_item,
            "wo_u": wo_u,
            "wo_i": wo_i,
            "b_user": b_user,
            "b_item": b_item,
            "b_out": b_out,
            "idx": idx_m,
        })
    return in_maps


def _run(inputs, trace=False):
    from concourse.bass_utils import run_bass_kernel_spmd

    nc = _build()
    in_maps = _make_in_maps(inputs)
    res = run_bass_kernel_spmd(
        nc, in_maps, core_ids=list(range(N_CORES)), trace=trace
    )
    parts = [res.results[k]["out"].reshape(-1)[:E_SH] for k in range(N_CORES)]
    full = np.concatenate(parts).reshape(E, 1).astype(np.float32)
    return full, res


def kernel(**inputs):
    full, _ = _run(inputs, trace=False)
    return full


# revision 22
# speedup vs baseline: 1.2999x; 1.0875x over previous
"""Trainium2 Bass kernel for ContextAwareArtRecSys (gnn_message_passing).

Math fold: the reference is
    score[e] = concat(z_u[src] @ Wu.T + bu, z_i[dst] @ Wi.T + bi) @ wo.T + bo
Everything after the gather is linear, so with
    vu = wo[:, :128] @ Wu          (256-vector)
    vi = wo[:, 128:] @ Wi          (256-vector)
    c  = wo[:, :128]@bu + wo[:, 128:]@bi + bo   (scalar)
we have score[e] = (z_u @ vu)[src] + (z_i @ vi)[dst] + c.

Sharding: edges are bucketed to cores BY DST RANGE (core k owns items
[k*12500, (k+1)*12500) and all edges pointing at them), sorted by dst and
packed into 17 rows x 3968 slots, whole dst-segments per row. Then:

  - item side needs NO gather and NO collective: each core expands its
    local item scores s_i across its edge slots with an indirect SCATTER
    of 12.5k values to segment-start slots followed by a masked-reset
    prefix scan (state = M*state + V) on DVE - exact, two instructions.
  - user side: compute the 6250-entry local user score shard, AllGather
    the 50k-entry table, then 17 indirect gathers (3968 descriptors each,
    output rows spread over the 16 SBUF AXI ports).

This cuts per-core random-access descriptors from 127k to 80k; indirect
DMA wall time is descriptor-bound (~3.5 ns/desc service, ~4 in flight),
so that is the dominant saving. z shards stream as 1.25 MB DMAs from a
host-swizzled [128, T*H] layout with the matvec on DVE (mul+reduce).
"""

import numpy as np

N_CORES = 8
N_USERS, N_ITEMS, E, H = 50000, 100000, 500000, 256
HALF = H // 2

U_SH = N_USERS // N_CORES          # 6250 users per core
I_SH = N_ITEMS // N_CORES          # 12500 items per core

U_TILES = 50                       # padded user row-tiles (6400 rows)
I_TILES = 100                      # padded item row-tiles (12800 rows)
CHUNK_T = 10                       # row-tiles per z DMA chunk (1.25 MB)
U_CHUNKS = U_TILES // CHUNK_T      # 5
I_CHUNKS = I_TILES // CHUNK_T      # 10

N_G = 17                           # user gather instructions
GCOLS = 31                         # idx columns per gather (31*128 = 3968)
RL = GCOLS * 128                   # 3968 slots per row / per gather
E_CAP = N_G * RL                   # 67456 edge slots per core

NSC = 4                            # scatter instructions
SC_N = 3200                        # values per scatter (25 cols * 128)
SCCOLS = SC_N // 128               # 25
OOB = 1 << 20                      # scatter index for "skip this value"

_CACHE = {}


def _build():
    if "nc" in _CACHE:
        return _CACHE["nc"]
    import concourse.bass as bass
    import concourse.tile as tile
    import concourse.mybir as mybir
    from concourse import bacc
    from concourse.bass import IndirectOffsetOnAxis

    f32 = mybir.dt.float32
    i32 = mybir.dt.int32

    nc = bacc.Bacc("TRN2", target_bir_lowering=False, debug=False,
                   num_devices=N_CORES, dynamic_dma_scratch_size=32768)

    zu = nc.dram_tensor("zu", [128, U_TILES * H], f32, kind="ExternalInput")
    zi = nc.dram_tensor("zi", [128, I_TILES * H], f32, kind="ExternalInput")
    w_user = nc.dram_tensor("w_user", [HALF, H], f32, kind="ExternalInput")
    w_item = nc.dram_tensor("w_item", [HALF, H], f32, kind="ExternalInput")
    wo_u = nc.dram_tensor("wo_u", [HALF, 1], f32, kind="ExternalInput")
    wo_i = nc.dram_tensor("wo_i", [HALF, 1], f32, kind="ExternalInput")
    b_user = nc.dram_tensor("b_user", [HALF, 1], f32, kind="ExternalInput")
    b_item = nc.dram_tensor("b_item", [HALF, 1], f32, kind="ExternalInput")
    b_out = nc.dram_tensor("b_out", [1, 1], f32, kind="ExternalInput")
    idxu = nc.dram_tensor("idxu", [128, N_G * GCOLS], i32, kind="ExternalInput")
    idxsc = nc.dram_tensor("idxsc", [128, NSC * SCCOLS], i32,
                           kind="ExternalInput")
    mrow = nc.dram_tensor("mrow", [N_G, RL], f32, kind="ExternalInput")
    out = nc.dram_tensor("out", [N_G, RL], f32, kind="ExternalOutput")

    s_uc = nc.dram_tensor("s_uc", [U_SH, 1], f32)
    s_ic = nc.dram_tensor("s_ic", [NSC * SC_N, 1], f32)
    s_uf = nc.dram_tensor("s_uf", [N_USERS, 1], f32, addr_space="Shared")
    dv = nc.dram_tensor("dv", [E_CAP, 1], f32)

    groups = [list(range(N_CORES))]

    with tile.TileContext(nc) as tc:
        with (
            tc.tile_pool(name="consts", bufs=1) as consts,
            tc.tile_pool(name="zpool", bufs=3) as zpool,
            tc.tile_pool(name="scpool", bufs=2) as scpool,
            tc.tile_pool(name="spool", bufs=1) as spool,
            tc.tile_pool(name="gpool", bufs=1) as gpool,
            tc.tile_pool(name="psum", bufs=2, space="PSUM") as psum,
        ):
            # ---- fold vu / vi / c on PE ----
            wu_t = consts.tile([HALF, H], f32)
            nc.sync.dma_start(wu_t[:], w_user.ap())
            wi_t = consts.tile([HALF, H], f32)
            nc.sync.dma_start(wi_t[:], w_item.ap())
            wou_t = consts.tile([HALF, 1], f32)
            nc.sync.dma_start(wou_t[:], wo_u.ap())
            woi_t = consts.tile([HALF, 1], f32)
            nc.sync.dma_start(woi_t[:], wo_i.ap())
            bu_t = consts.tile([HALF, 1], f32)
            nc.sync.dma_start(bu_t[:], b_user.ap())
            bi_t = consts.tile([HALF, 1], f32)
            nc.sync.dma_start(bi_t[:], b_item.ap())
            bo_t = consts.tile([1, 1], f32)
            nc.sync.dma_start(bo_t[:], b_out.ap())

            # load gather/scatter indices + scan mask early
            idxu_t = gpool.tile([128, N_G * GCOLS], i32)
            nc.sync.dma_start(idxu_t[:], idxu.ap())
            idxsc_t = gpool.tile([128, NSC * SCCOLS], i32)
            nc.sync.dma_start(idxsc_t[:], idxsc.ap())
            m_t = gpool.tile([N_G, RL], f32)
            nc.sync.dma_start(m_t[:], mrow.ap())

            # zero the scatter destination early (slab shared with sc_t)
            z0_t = gpool.tile([N_G, RL], f32, tag="zsc")
            nc.vector.memset(z0_t[:], 0.0)
            nc.sync.dma_start(
                dv.ap().rearrange("(a b) one -> a (b one)", a=N_G), z0_t[:]
            )

            # replicate wo halves across the free dim: rep[k, m] = wo[k]
            ones_kk = consts.tile([HALF, HALF], f32)
            nc.vector.memset(ones_kk[:], 1.0)
            wou_rep = consts.tile([HALF, HALF], f32)
            nc.vector.tensor_scalar_mul(wou_rep[:], ones_kk[:], wou_t[:])
            woi_rep = consts.tile([HALF, HALF], f32)
            nc.vector.tensor_scalar_mul(woi_rep[:], ones_kk[:], woi_t[:])

            # vu/vi broadcast across all 128 partitions: [128, H] PSUM
            vu_ps = psum.tile([HALF, H], f32, tag="vps")
            nc.tensor.matmul(vu_ps[:], wou_rep[:], wu_t[:], start=True, stop=True)
            vu_t = consts.tile([HALF, H], f32)
            nc.vector.tensor_copy(vu_t[:], vu_ps[:])
            vi_ps = psum.tile([HALF, H], f32, tag="vps")
            nc.tensor.matmul(vi_ps[:], woi_rep[:], wi_t[:], start=True, stop=True)
            vi_t = consts.tile([HALF, H], f32)
            nc.vector.tensor_copy(vi_t[:], vi_ps[:])

            # c = wo_u . b_user + wo_i . b_item + b_out, broadcast to [128,1]
            ones_k1 = consts.tile([HALF, 128], f32)
            nc.vector.memset(ones_k1[:], 1.0)
            cu_ps = psum.tile([128, 1], f32, tag="cps")
            bub = consts.tile([HALF, 128], f32)
            nc.vector.tensor_scalar_mul(bub[:], ones_k1[:], bu_t[:])
            bib = consts.tile([HALF, 128], f32)
            nc.vector.tensor_scalar_mul(bib[:], ones_k1[:], bi_t[:])
            nc.tensor.matmul(cu_ps[:], bub[:], wou_t[:], start=True, stop=False)
            nc.tensor.matmul(cu_ps[:], bib[:], woi_t[:], start=False, stop=False)
            nc.tensor.matmul(
                cu_ps[:], ones_k1[0:1, :], bo_t[:], start=False, stop=True
            )
            c_t = consts.tile([128, 1], f32)
            nc.vector.tensor_copy(c_t[:], cu_ps[:])

            # ---- item z phase first: feeds the local expansion ----
            si_sb = spool.tile([128, I_TILES], f32)
            for ch in range(I_CHUNKS):
                zt = zpool.tile([128, CHUNK_T * H], f32, tag="z")
                nc.sync.dma_start(
                    zt[:], zi.ap()[:, ch * CHUNK_T * H:(ch + 1) * CHUNK_T * H]
                )
                for t in range(CHUNK_T):
                    pr = scpool.tile([128, H], f32, tag="scr")
                    nc.vector.tensor_mul(pr[:], zt[:, t * H:(t + 1) * H], vi_t[:])
                    nc.vector.reduce_sum(
                        si_sb[:, ch * CHUNK_T + t:ch * CHUNK_T + t + 1],
                        pr[:],
                        axis=mybir.AxisListType.X,
                    )

            # store item scores node-major, reload as a single-partition row
            n_full_i = I_SH // 128              # 97
            rem_i = I_SH - n_full_i * 128       # 84
            nc.sync.dma_start(
                s_ic.ap()[: n_full_i * 128, :].rearrange(
                    "(t p) one -> p (t one)", p=128
                ),
                si_sb[:, :n_full_i],
            )
            nc.sync.dma_start(
                s_ic.ap()[n_full_i * 128:I_SH, :],
                si_sb[:rem_i, n_full_i:n_full_i + 1],
            )
            sv_t = gpool.tile([NSC, SC_N], f32)
            nc.sync.dma_start(
                sv_t[:],
                s_ic.ap().rearrange("(a b) one -> a (b one)", a=NSC),
            )

            # ---- user z phase -> local shard of the user table ----
            su_sb = spool.tile([128, U_TILES], f32)
            for ch in range(U_CHUNKS):
                zt = zpool.tile([128, CHUNK_T * H], f32, tag="z")
                nc.sync.dma_start(
                    zt[:], zu.ap()[:, ch * CHUNK_T * H:(ch + 1) * CHUNK_T * H]
                )
                for t in range(CHUNK_T):
                    pr = scpool.tile([128, H], f32, tag="scr")
                    nc.vector.tensor_mul(pr[:], zt[:, t * H:(t + 1) * H], vu_t[:])
                    nc.vector.reduce_sum(
                        su_sb[:, ch * CHUNK_T + t:ch * CHUNK_T + t + 1],
                        pr[:],
                        axis=mybir.AxisListType.X,
                    )
            # add folded constant c into the user table
            nc.vector.tensor_scalar_add(su_sb[:], su_sb[:], c_t[:])

            n_full_u = U_SH // 128              # 48 full tiles
            rem_u = U_SH - n_full_u * 128       # 106
            nc.sync.dma_start(
                s_uc.ap()[: n_full_u * 128, :].rearrange(
                    "(t p) one -> p (t one)", p=128
                ),
                su_sb[:, :n_full_u],
            )
            nc.sync.dma_start(
                s_uc.ap()[n_full_u * 128:, :],
                su_sb[:rem_u, n_full_u:n_full_u + 1],
            )
            nc.gpsimd.collective_compute(
                "AllGather", mybir.AluOpType.bypass,
                replica_groups=groups, ins=[s_uc.ap()], outs=[s_uf.ap()],
            )

            # ---- item expansion: scatter s_i to segment starts in dv ----
            for t in range(NSC):
                nc.gpsimd.indirect_dma_start(
                    out=dv.ap(),
                    out_offset=IndirectOffsetOnAxis(
                        ap=idxsc_t[:, t * SCCOLS:(t + 1) * SCCOLS], axis=0
                    ),
                    in_=sv_t[t:t + 1, :].rearrange(
                        "one (c x) -> one c x", x=1
                    ),
                    in_offset=None,
                    bounds_check=E_CAP - 1,
                    oob_is_err=False,
                )

            # ---- user gathers: gather g -> partition row 7g so drains
            # spread over the 16 AXI ports ----
            gu_t = gpool.tile([128, RL], f32)
            for g in range(N_G):
                nc.gpsimd.indirect_dma_start(
                    out=gu_t[7 * g:7 * g + 1, :].rearrange(
                        "one (c x) -> one c x", x=1
                    ),
                    out_offset=None,
                    in_=s_uf.ap(),
                    in_offset=IndirectOffsetOnAxis(
                        ap=idxu_t[:, g * GCOLS:(g + 1) * GCOLS], axis=0
                    ),
                )

            # ---- expansion scan: state = M*state + V per slot row ----
            dv_t = gpool.tile([N_G, RL], f32, tag="dvg")
            nc.sync.dma_start(
                dv_t[:], dv.ap().rearrange("(a b) one -> a (b one)", a=N_G)
            )
            a_t = gpool.tile([N_G, RL], f32)
            nc.vector.tensor_tensor_scan(
                a_t[:], m_t[:], dv_t[:], 0.0,
                mybir.AluOpType.mult, mybir.AluOpType.add,
            )

            # ---- compact gather rows, add item expansion, store ----
            guc = gpool.tile([N_G, RL], f32, tag="dvg")
            nc.sync.dma_start(guc[0:9, :], gu_t[0:57:7, :])
            nc.sync.dma_start(guc[9:N_G, :], gu_t[63:113:7, :])
            sc_t = gpool.tile([N_G, RL], f32, tag="zsc")
            nc.vector.tensor_add(sc_t[:], guc[:], a_t[:])
            nc.sync.dma_start(out.ap(), sc_t[:])

    nc.compile()
    _CACHE["nc"] = nc
    return nc


def _wrap_pf(vals, cols):
    """Partition-fastest wrap: stream element i -> tile[i%128, i//128]."""
    n = len(vals)
    assert n % 128 == 0 and n // 128 == cols
    return np.ascontiguousarray(vals.reshape(cols, 128).T)


def _swizzle_z(rows, tiles):
    """rows [n, H] -> [128, tiles*H] with column block t = rows[t*128:(t+1)*128]."""
    n = rows.shape[0]
    padded = np.zeros((tiles * 128, H), dtype=np.float32)
    padded[:n] = rows
    return np.ascontiguousarray(
        padded.reshape(tiles, 128, H).transpose(1, 0, 2).reshape(128, tiles * H)
    )


def _pack_core(src_k, dst_k, base_item):
    """Pack one core's edges (sorted by dst) into N_G rows of RL slots,
    whole dst-segments per row.

    Returns (idxu [128, N_G*GCOLS], idxsc_dest [I_SH padded to NSC*SC_N],
    mask [N_G, RL], eids [N_G, RL] original-edge-id per slot or -1).
    """
    order = np.argsort(dst_k, kind="stable")
    dsts = dst_k[order]
    # segment boundaries in the sorted list
    seg_starts = np.flatnonzero(np.r_[True, dsts[1:] != dsts[:-1]])
    seg_ends = np.r_[seg_starts[1:], len(dsts)]

    idxu_lin = np.zeros(E_CAP, dtype=np.int32)
    eids = np.full((N_G, RL), -1, dtype=np.int64)
    mask = np.ones((N_G, RL), dtype=np.float32)
    dest = np.full(NSC * SC_N, OOB, dtype=np.int32)

    row, pos = 0, 0
    for s, epos in zip(seg_starts, seg_ends):
        seg_len = epos - s
        if pos + seg_len > RL:
            row += 1
            pos = 0
            assert row < N_G, "edge packing overflow"
        item_local = dsts[s] - base_item
        slot0 = row * RL + pos
        dest[item_local] = slot0
        mask[row, pos] = 0.0
        eids[row, pos:pos + seg_len] = order[s:epos]
        idxu_lin[slot0:slot0 + seg_len] = src_k[order[s:epos]]
        pos += seg_len

    idxu = np.empty((128, N_G * GCOLS), dtype=np.int32)
    for g in range(N_G):
        idxu[:, g * GCOLS:(g + 1) * GCOLS] = _wrap_pf(
            idxu_lin[g * RL:(g + 1) * RL], GCOLS
        )
    return idxu, dest, mask, eids


def _make_in_maps(inputs):
    z_user = np.asarray(inputs["z_user"], dtype=np.float32)
    z_item = np.asarray(inputs["z_item"], dtype=np.float32)
    src = np.asarray(inputs["edge_src"]).astype(np.int32)
    dst = np.asarray(inputs["edge_dst"]).astype(np.int32)
    w_user = np.asarray(inputs["w_user"], dtype=np.float32)
    w_item = np.asarray(inputs["w_item"], dtype=np.float32)
    b_user = np.asarray(inputs["b_user"], dtype=np.float32).reshape(HALF, 1)
    b_item = np.asarray(inputs["b_item"], dtype=np.float32).reshape(HALF, 1)
    w_out = np.asarray(inputs["w_out"], dtype=np.float32)
    b_out = np.asarray(inputs["b_out"], dtype=np.float32).reshape(1, 1)
    wo_u = w_out[0, :HALF].reshape(HALF, 1).copy()
    wo_i = w_out[0, HALF:].reshape(HALF, 1).copy()

    bucket = dst // I_SH
    in_maps = []
    slot_eids = []
    for k in range(N_CORES):
        sel = np.flatnonzero(bucket == k)
        idxu_m, dest, mask, eids = _pack_core(
            src[sel], dst[sel], k * I_SH
        )
        # eids are positions within sel; map to global edge ids
        eids_g = np.where(eids >= 0, sel[np.clip(eids, 0, None)], -1)
        slot_eids.append(eids_g)
        idxsc_m = np.empty((128, NSC * SCCOLS), dtype=np.int32)
        for t in range(NSC):
            idxsc_m[:, t * SCCOLS:(t + 1) * SCCOLS] = _wrap_pf(
                dest[t * SC_N:(t + 1) * SC_N], SCCOLS
            )
        in_maps.append({
            "zu": _swizzle_z(z_user[k * U_SH:(k + 1) * U_SH], U_TILES),
            "zi": _swizzle_z(z_item[k * I_SH:(k + 1) * I_SH], I_TILES),
            "w_user": w_user,
            "w_item": w_item,
            "wo_u": wo_u,
            "wo_i": wo_i,
            "b_user": b_user,
            "b_item": b_item,
            "b_out": b_out,
            "idxu": idxu_m,
            "idxsc": idxsc_m,
            "mrow": mask,
        })
    return in_maps, slot_eids


def _run(inputs, trace=False):
    from concourse.bass_utils import run_bass_kernel_spmd

    nc = _build()
    in_maps, slot_eids = _make_in_maps(inputs)
    res = run_bass_kernel_spmd(
        nc, in_maps, core_ids=list(range(N_CORES)), trace=trace
    )
    full = np.empty(E, dtype=np.float32)
    for k in range(N_CORES):
        vals = res.results[k]["out"].reshape(-1)
        eids = slot_eids[k].reshape(-1)
        real = eids >= 0
        full[eids[real]] = vals[real]
    return full.reshape(E, 1), res


def kernel(**inputs):
    full, _ = _run(inputs, trace=False)
    return full


# revision 23
# speedup vs baseline: 1.7350x; 1.3348x over previous
"""Trainium2 Bass kernel for ContextAwareArtRecSys (gnn_message_passing).

Math fold: the reference is
    score[e] = concat(z_u[src] @ Wu.T + bu, z_i[dst] @ Wi.T + bi) @ wo.T + bo
Everything after the gather is linear, so with
    vu = wo[:, :128] @ Wu          (256-vector)
    vi = wo[:, 128:] @ Wi          (256-vector)
    c  = wo[:, :128]@bu + wo[:, 128:]@bi + bo   (scalar)
we have score[e] = (z_u @ vu)[src] + (z_i @ vi)[dst] + c.

Sharding: edges are bucketed to cores BY DST RANGE (core k owns items
[k*12500, (k+1)*12500) and all edges pointing at them), sorted by dst and
packed into 17 rows x 3968 slots, whole dst-segments per row. Then:

  - item side needs NO gather and NO collective: each core expands its
    local item scores s_i across its edge slots with an indirect SCATTER
    of 12.5k values to segment-start slots followed by a masked-reset
    prefix scan (state = M*state + V) on DVE - exact, two instructions.
  - user side: compute the 6250-entry local user score shard, AllGather
    the 50k-entry table, then 17 indirect gathers (3968 descriptors each).

Indirect-DMA service is the wall (the SWDGE queue's SDMA engines process
descriptors at ~7 ns each, ~2 engines per queue), so the 21 indirect DMAs
are spread round-robin over 4 SWDGE queues (independent engine sets) and
the score tables are stored partition-major (one big descriptor per
partition instead of 12k 4-byte descriptors for a node-major transpose) -
the gather/scatter index values absorb the layout on the host.
"""

import numpy as np

N_CORES = 8
N_USERS, N_ITEMS, E, H = 50000, 100000, 500000, 256
HALF = H // 2

U_SH = N_USERS // N_CORES          # 6250 users per core
I_SH = N_ITEMS // N_CORES          # 12500 items per core

U_TILES = 50                       # padded user row-tiles (6400 rows)
I_TILES = 100                      # padded item row-tiles (12800 rows)
CHUNK_T = 10                       # row-tiles per z DMA chunk (1.25 MB)
U_CHUNKS = U_TILES // CHUNK_T      # 5
I_CHUNKS = I_TILES // CHUNK_T      # 10
U_PAD = U_TILES * 128              # 6400 stored user scores per core
I_PAD = I_TILES * 128              # 12800 stored item scores per core

N_G = 17                           # user gather instructions
GCOLS = 31                         # idx columns per gather (31*128 = 3968)
RL = GCOLS * 128                   # 3968 slots per row / per gather
E_CAP = N_G * RL                   # 67456 edge slots per core

NSC = 4                            # scatter instructions
SC_N = I_PAD // NSC                # 3200 values per scatter
SCCOLS = SC_N // 128               # 25
OOB = 1 << 20                      # scatter index for "skip this value"
N_QUEUES = 4                       # SWDGE queues for indirect DMAs

_CACHE = {}


def _build():
    if "nc" in _CACHE:
        return _CACHE["nc"]
    import concourse.bass as bass
    import concourse.tile as tile
    import concourse.mybir as mybir
    from concourse import bacc
    from concourse.bass import IndirectOffsetOnAxis

    f32 = mybir.dt.float32
    i32 = mybir.dt.int32

    nc = bacc.Bacc("TRN2", target_bir_lowering=False, debug=False,
                   num_devices=N_CORES, dynamic_dma_scratch_size=32768,
                   num_swdge_queues=N_QUEUES)

    qnames = ["qPoolDynamic"] + [f"qPoolDynamic{i}" for i in range(1, N_QUEUES)]

    zu = nc.dram_tensor("zu", [128, U_TILES * H], f32, kind="ExternalInput")
    zi = nc.dram_tensor("zi", [128, I_TILES * H], f32, kind="ExternalInput")
    w_user = nc.dram_tensor("w_user", [HALF, H], f32, kind="ExternalInput")
    w_item = nc.dram_tensor("w_item", [HALF, H], f32, kind="ExternalInput")
    wo_u = nc.dram_tensor("wo_u", [HALF, 1], f32, kind="ExternalInput")
    wo_i = nc.dram_tensor("wo_i", [HALF, 1], f32, kind="ExternalInput")
    b_user = nc.dram_tensor("b_user", [HALF, 1], f32, kind="ExternalInput")
    b_item = nc.dram_tensor("b_item", [HALF, 1], f32, kind="ExternalInput")
    b_out = nc.dram_tensor("b_out", [1, 1], f32, kind="ExternalInput")
    idxu = nc.dram_tensor("idxu", [128, N_G * GCOLS], i32, kind="ExternalInput")
    idxsc = nc.dram_tensor("idxsc", [128, NSC * SCCOLS], i32,
                           kind="ExternalInput")
    mrow = nc.dram_tensor("mrow", [N_G, RL], f32, kind="ExternalInput")
    out = nc.dram_tensor("out", [N_G, RL], f32, kind="ExternalOutput")

    s_uc = nc.dram_tensor("s_uc", [U_PAD, 1], f32)
    s_ic = nc.dram_tensor("s_ic", [I_PAD, 1], f32)
    s_uf = nc.dram_tensor("s_uf", [N_CORES * U_PAD, 1], f32,
                          addr_space="Shared")
    dv = nc.dram_tensor("dv", [E_CAP, 1], f32)

    groups = [list(range(N_CORES))]

    with tile.TileContext(nc) as tc:
        with (
            tc.tile_pool(name="consts", bufs=1) as consts,
            tc.tile_pool(name="zpool", bufs=3) as zpool,
            tc.tile_pool(name="scpool", bufs=2) as scpool,
            tc.tile_pool(name="spool", bufs=1) as spool,
            tc.tile_pool(name="gpool", bufs=1) as gpool,
            tc.tile_pool(name="psum", bufs=2, space="PSUM") as psum,
        ):
            # ---- fold vu / vi / c on PE ----
            wu_t = consts.tile([HALF, H], f32)
            nc.sync.dma_start(wu_t[:], w_user.ap())
            wi_t = consts.tile([HALF, H], f32)
            nc.sync.dma_start(wi_t[:], w_item.ap())
            wou_t = consts.tile([HALF, 1], f32)
            nc.sync.dma_start(wou_t[:], wo_u.ap())
            woi_t = consts.tile([HALF, 1], f32)
            nc.sync.dma_start(woi_t[:], wo_i.ap())
            bu_t = consts.tile([HALF, 1], f32)
            nc.sync.dma_start(bu_t[:], b_user.ap())
            bi_t = consts.tile([HALF, 1], f32)
            nc.sync.dma_start(bi_t[:], b_item.ap())
            bo_t = consts.tile([1, 1], f32)
            nc.sync.dma_start(bo_t[:], b_out.ap())

            # replicate wo halves across the free dim: rep[k, m] = wo[k]
            ones_kk = consts.tile([HALF, HALF], f32)
            nc.vector.memset(ones_kk[:], 1.0)
            wou_rep = consts.tile([HALF, HALF], f32)
            nc.vector.tensor_scalar_mul(wou_rep[:], ones_kk[:], wou_t[:])
            woi_rep = consts.tile([HALF, HALF], f32)
            nc.vector.tensor_scalar_mul(woi_rep[:], ones_kk[:], woi_t[:])

            # vu/vi broadcast across all 128 partitions: [128, H] PSUM
            vu_ps = psum.tile([HALF, H], f32, tag="vps")
            nc.tensor.matmul(vu_ps[:], wou_rep[:], wu_t[:], start=True, stop=True)
            vu_t = consts.tile([HALF, H], f32)
            nc.vector.tensor_copy(vu_t[:], vu_ps[:])
            vi_ps = psum.tile([HALF, H], f32, tag="vps")
            nc.tensor.matmul(vi_ps[:], woi_rep[:], wi_t[:], start=True, stop=True)
            vi_t = consts.tile([HALF, H], f32)
            nc.vector.tensor_copy(vi_t[:], vi_ps[:])

            # c = wo_u . b_user + wo_i . b_item + b_out, broadcast to [128,1]
            ones_k1 = consts.tile([HALF, 128], f32)
            nc.vector.memset(ones_k1[:], 1.0)
            cu_ps = psum.tile([128, 1], f32, tag="cps")
            bub = consts.tile([HALF, 128], f32)
            nc.vector.tensor_scalar_mul(bub[:], ones_k1[:], bu_t[:])
            bib = consts.tile([HALF, 128], f32)
            nc.vector.tensor_scalar_mul(bib[:], ones_k1[:], bi_t[:])
            nc.tensor.matmul(cu_ps[:], bub[:], wou_t[:], start=True, stop=False)
            nc.tensor.matmul(cu_ps[:], bib[:], woi_t[:], start=False, stop=False)
            nc.tensor.matmul(
                cu_ps[:], ones_k1[0:1, :], bo_t[:], start=False, stop=True
            )
            c_t = consts.tile([128, 1], f32)
            nc.vector.tensor_copy(c_t[:], cu_ps[:])

            # ---- user z phase first: its table gates the AllGather ----
            su_sb = spool.tile([128, U_TILES], f32)
            for ch in range(U_CHUNKS):
                zt = zpool.tile([128, CHUNK_T * H], f32, tag="z")
                nc.sync.dma_start(
                    zt[:], zu.ap()[:, ch * CHUNK_T * H:(ch + 1) * CHUNK_T * H]
                )
                for t in range(CHUNK_T):
                    pr = scpool.tile([128, H], f32, tag="scr")
                    nc.vector.tensor_mul(pr[:], zt[:, t * H:(t + 1) * H], vu_t[:])
                    nc.vector.reduce_sum(
                        su_sb[:, ch * CHUNK_T + t:ch * CHUNK_T + t + 1],
                        pr[:],
                        axis=mybir.AxisListType.X,
                    )
            # add folded constant c into the user table
            nc.vector.tensor_scalar_add(su_sb[:], su_sb[:], c_t[:])

            # store partition-major (one fat descriptor per partition) and
            # AllGather; the gather indices absorb the layout.
            nc.sync.dma_start(
                s_uc.ap().rearrange("(p t) one -> p (t one)", p=128),
                su_sb[:],
            )
            nc.gpsimd.collective_compute(
                "AllGather", mybir.AluOpType.bypass,
                replica_groups=groups, ins=[s_uc.ap()], outs=[s_uf.ap()],
            )

            # ---- user gathers, round-robin over the SWDGE queues ----
            idxu_t = gpool.tile([128, N_G * GCOLS], i32)
            nc.sync.dma_start(idxu_t[:], idxu.ap())
            gu_t = gpool.tile([128, RL], f32)
            for g in range(N_G):
                inst = nc.gpsimd.indirect_dma_start(
                    out=gu_t[7 * g:7 * g + 1, :].rearrange(
                        "one (c x) -> one c x", x=1
                    ),
                    out_offset=None,
                    in_=s_uf.ap(),
                    in_offset=IndirectOffsetOnAxis(
                        ap=idxu_t[:, g * GCOLS:(g + 1) * GCOLS], axis=0
                    ),
                )
                inst.ins.queue = qnames[g % N_QUEUES]

            # ---- item z phase ----
            si_sb = spool.tile([128, I_TILES], f32)
            for ch in range(I_CHUNKS):
                zt = zpool.tile([128, CHUNK_T * H], f32, tag="z")
                nc.sync.dma_start(
                    zt[:], zi.ap()[:, ch * CHUNK_T * H:(ch + 1) * CHUNK_T * H]
                )
                for t in range(CHUNK_T):
                    pr = scpool.tile([128, H], f32, tag="scr")
                    nc.vector.tensor_mul(pr[:], zt[:, t * H:(t + 1) * H], vi_t[:])
                    nc.vector.reduce_sum(
                        si_sb[:, ch * CHUNK_T + t:ch * CHUNK_T + t + 1],
                        pr[:],
                        axis=mybir.AxisListType.X,
                    )

            # store item scores partition-major, reload as 4 scatter rows
            nc.sync.dma_start(
                s_ic.ap().rearrange("(p t) one -> p (t one)", p=128),
                si_sb[:],
            )
            sv_t = gpool.tile([NSC, SC_N], f32)
            nc.sync.dma_start(
                sv_t[:],
                s_ic.ap().rearrange("(a b) one -> a (b one)", a=NSC),
            )

            # zero the scatter destination (slab shared with sc_t)
            idxsc_t = gpool.tile([128, NSC * SCCOLS], i32)
            nc.sync.dma_start(idxsc_t[:], idxsc.ap())
            m_t = gpool.tile([N_G, RL], f32)
            nc.sync.dma_start(m_t[:], mrow.ap())
            z0_t = gpool.tile([N_G, RL], f32, tag="zsc")
            nc.vector.memset(z0_t[:], 0.0)
            nc.sync.dma_start(
                dv.ap().rearrange("(a b) one -> a (b one)", a=N_G), z0_t[:]
            )

            # ---- item expansion: scatter s_i to segment starts in dv ----
            for t in range(NSC):
                inst = nc.gpsimd.indirect_dma_start(
                    out=dv.ap(),
                    out_offset=IndirectOffsetOnAxis(
                        ap=idxsc_t[:, t * SCCOLS:(t + 1) * SCCOLS], axis=0
                    ),
                    in_=sv_t[t:t + 1, :].rearrange(
                        "one (c x) -> one c x", x=1
                    ),
                    in_offset=None,
                    bounds_check=E_CAP - 1,
                    oob_is_err=False,
                )
                inst.ins.queue = qnames[t % N_QUEUES]

            # ---- compact gather rows 0,7,...,112 -> 17 partitions ----
            guc = gpool.tile([N_G, RL], f32, tag="dvg2")
            nc.sync.dma_start(guc[0:9, :], gu_t[0:57:7, :])
            nc.sync.dma_start(guc[9:N_G, :], gu_t[63:113:7, :])

            # ---- expansion scan: state = M*state + V per slot row ----
            dv_t = gpool.tile([N_G, RL], f32)
            nc.sync.dma_start(
                dv_t[:], dv.ap().rearrange("(a b) one -> a (b one)", a=N_G)
            )
            a_t = gpool.tile([N_G, RL], f32)
            nc.vector.tensor_tensor_scan(
                a_t[:], m_t[:], dv_t[:], 0.0,
                mybir.AluOpType.mult, mybir.AluOpType.add,
            )

            sc_t = gpool.tile([N_G, RL], f32, tag="zsc")
            nc.vector.tensor_add(sc_t[:], guc[:], a_t[:])
            nc.sync.dma_start(out.ap(), sc_t[:])

    nc.compile()
    _CACHE["nc"] = nc
    return nc


def _wrap_pf(vals, cols):
    """Partition-fastest wrap: stream element i -> tile[i%128, i//128]."""
    n = len(vals)
    assert n % 128 == 0 and n // 128 == cols
    return np.ascontiguousarray(vals.reshape(cols, 128).T)


def _swizzle_z(rows, tiles):
    """rows [n, H] -> [128, tiles*H] with column block t = rows[t*128:(t+1)*128]."""
    n = rows.shape[0]
    padded = np.zeros((tiles * 128, H), dtype=np.float32)
    padded[:n] = rows
    return np.ascontiguousarray(
        padded.reshape(tiles, 128, H).transpose(1, 0, 2).reshape(128, tiles * H)
    )


def _uidx(u):
    """Global user id -> position in the partition-major AllGathered table."""
    c, n = u // U_SH, u % U_SH
    return c * U_PAD + (n % 128) * U_TILES + n // 128


def _pack_core(src_k, dst_k, base_item):
    """Pack one core's edges (sorted by dst) into N_G rows of RL slots,
    whole dst-segments per row.

    Returns (idxu [128, N_G*GCOLS], dest_stream [NSC*SC_N], mask [N_G, RL],
    eids [N_G, RL] position-in-src_k per slot or -1).
    """
    order = np.argsort(dst_k, kind="stable")
    dsts = dst_k[order]
    seg_starts = np.flatnonzero(np.r_[True, dsts[1:] != dsts[:-1]])
    seg_ends = np.r_[seg_starts[1:], len(dsts)]

    idxu_lin = np.zeros(E_CAP, dtype=np.int32)
    eids = np.full((N_G, RL), -1, dtype=np.int64)
    mask = np.ones((N_G, RL), dtype=np.float32)
    dest_node = np.full(I_SH, OOB, dtype=np.int32)

    row, pos = 0, 0
    for s, epos in zip(seg_starts, seg_ends):
        seg_len = epos - s
        if pos + seg_len > RL:
            row += 1
            pos = 0
            assert row < N_G, "edge packing overflow"
        item_local = dsts[s] - base_item
        slot0 = row * RL + pos
        dest_node[item_local] = slot0
        mask[row, pos] = 0.0
        eids[row, pos:pos + seg_len] = order[s:epos]
        idxu_lin[slot0:slot0 + seg_len] = _uidx(src_k[order[s:epos]])
        pos += seg_len

    idxu = np.empty((128, N_G * GCOLS), dtype=np.int32)
    for g in range(N_G):
        idxu[:, g * GCOLS:(g + 1) * GCOLS] = _wrap_pf(
            idxu_lin[g * RL:(g + 1) * RL], GCOLS
        )
    # scatter value stream j reads s_ic flat j = p*I_TILES + t, which holds
    # the score of local item n = 128*t + p
    j = np.arange(NSC * SC_N)
    n = 128 * (j % I_TILES) + j // I_TILES
    dest = np.where(n < I_SH, dest_node[np.minimum(n, I_SH - 1)], OOB)
    return idxu, dest.astype(np.int32), mask, eids


def _make_in_maps(inputs):
    z_user = np.asarray(inputs["z_user"], dtype=np.float32)
    z_item = np.asarray(inputs["z_item"], dtype=np.float32)
    src = np.asarray(inputs["edge_src"]).astype(np.int32)
    dst = np.asarray(inputs["edge_dst"]).astype(np.int32)
    w_user = np.asarray(inputs["w_user"], dtype=np.float32)
    w_item = np.asarray(inputs["w_item"], dtype=np.float32)
    b_user = np.asarray(inputs["b_user"], dtype=np.float32).reshape(HALF, 1)
    b_item = np.asarray(inputs["b_item"], dtype=np.float32).reshape(HALF, 1)
    w_out = np.asarray(inputs["w_out"], dtype=np.float32)
    b_out = np.asarray(inputs["b_out"], dtype=np.float32).reshape(1, 1)
    wo_u = w_out[0, :HALF].reshape(HALF, 1).copy()
    wo_i = w_out[0, HALF:].reshape(HALF, 1).copy()

    bucket = dst // I_SH
    in_maps = []
    slot_eids = []
    for k in range(N_CORES):
        sel = np.flatnonzero(bucket == k)
        idxu_m, dest, mask, eids = _pack_core(src[sel], dst[sel], k * I_SH)
        eids_g = np.where(eids >= 0, sel[np.clip(eids, 0, None)], -1)
        slot_eids.append(eids_g)
        idxsc_m = np.empty((128, NSC * SCCOLS), dtype=np.int32)
        for t in range(NSC):
            idxsc_m[:, t * SCCOLS:(t + 1) * SCCOLS] = _wrap_pf(
                dest[t * SC_N:(t + 1) * SC_N], SCCOLS
            )
        in_maps.append({
            "zu": _swizzle_z(z_user[k * U_SH:(k + 1) * U_SH], U_TILES),
            "zi": _swizzle_z(z_item[k * I_SH:(k + 1) * I_SH], I_TILES),
            "w_user": w_user,
            "w_item": w_item,
            "wo_u": wo_u,
            "wo_i": wo_i,
            "b_user": b_user,
            "b_item": b_item,
            "b_out": b_out,
            "idxu": idxu_m,
            "idxsc": idxsc_m,
            "mrow": mask,
        })
    return in_maps, slot_eids


def _run(inputs, trace=False):
    from concourse.bass_utils import run_bass_kernel_spmd

    nc = _build()
    in_maps, slot_eids = _make_in_maps(inputs)
    res = run_bass_kernel_spmd(
        nc, in_maps, core_ids=list(range(N_CORES)), trace=trace
    )
    full = np.empty(E, dtype=np.float32)
    for k in range(N_CORES):
        vals = res.results[k]["out"].reshape(-1)
        eids = slot_eids[k].reshape(-1)
        real = eids >= 0
        full[eids[real]] = vals[real]
    return full.reshape(E, 1), res


def kernel(**inputs):
    full, _ = _run(inputs, trace=False)
    return full
